# revision 10
# baseline (speedup 1.0000x reference)
"""MoE conditional feed-forward (T=1024, D=1024, H=2048, E=32, K=2) on 8 trn2 cores.

Sharding: expert-parallel, E/8 = 4 experts per core. Host gathers the tokens
routed to each expert (dispatch), the device runs the expert FFNs on padded
128-token blocks, the host scatters results back (combine).

v3: weights are int8 in DRAM (per-channel symmetric quantization, host-side),
dequantized to fp16 on-chip across THREE engines (DVE + ACT + Pool), and fed
to the PE as the MOVING matmul operand. The moving operand streams at
128 elem/cycle @ 2.4 GHz (~307 G elem/s); v2 made weights the stationary
operand, which loads at only ~1.2 GHz column rate and serializes the PE
sequencer with 1152 LDWEIGHTS+MATMUL pairs (~118 ns/pair measured). v3 issues
~96 matmuls per item with rarely-changing stationary tiles.

Quantization axes (scale constant along each SBUF partition row):
  Wgu: scale per (e, d)  — partition dim of the gu stationary/moving layout
  Wd:  scale per (e, h)  — partition dim of the down moving layout

Device dataflow per (expert, 128-token block) work item:
  gu stage, 4 feature chunks fc of (512 gate + 512 up):
    ps_g/ps_u[t, 512] += xt[:,dc,:].T @ w16[:, dc, g/u, :]   (dc = 0..7)
    inter_fc[t, 512] = silu(ps_g) * ps_u                      (ACT + DVE)
    interT[h%128, fc*4+k, t] = PE-transpose(inter_fc)         (4x 128x128)
  down stage, 2 halves hh of 8 h-chunks:
    ps_o[t, 1024] += interT[:, hc, :].T @ wd16[:, hcl, :]     (hc = 0..15)
  out[t, 1024] = fp16(ps_o)                                   (ACT) -> DMA

DMA: big per-chunk transfers (1 MiB) split across both HWDGE rings
(sync + scalar) so transfer completion latency overlaps.
"""

import numpy as np

T, D, H, E, K = 1024, 1024, 2048, 32, 2
NCORES = 8
EPC = E // NCORES  # experts per core
C = 128            # token capacity per work item

_CACHE: dict = {}


def _build(
    nw: int,
    cap: int = C,
    loop_n: int | None = None,
    rep: int = 1,
    probe: str = "",
    gu_split: tuple = (4, 2, 2),
    wd_split: tuple = (4, 2, 2),
):
    """Build + compile the SPMD Bass program for nw work items per core.

    gu_split / wd_split: (n_dve, n_act, n_pool) assignment of the 8 dequant
    slices of each wgu / wd chunk to the vector / scalar / pool engines.
    probe: "dma" (DMA only), "deqonly" (DMA + dequant), "nodeq" (DMA +
    matmul, dequant strided to ~0 cost).
    """
    import concourse.mybir as mybir
    import concourse.tile as tile
    from concourse import bacc
    from concourse.masks import make_identity

    assert cap == 128, "v3 kernel is specialized to cap=128"
    assert sum(gu_split) == 8 and sum(wd_split) == 8

    do_dequant = probe not in ("dma", "nodeq")
    do_compute = probe not in ("dma", "deqonly")

    i8 = mybir.dt.int8
    f16 = mybir.dt.float16
    f32 = mybir.dt.float32

    nc = bacc.Bacc(
        "TRN2",
        target_bir_lowering=False,
        debug=False,
        enable_asserts=False,
        num_devices=NCORES,
    )

    # Per-core DRAM parameters (host pre-arranged, partition-major):
    #   xt  : [nw, 128, 8, 128] f16   xt[j, dp, dc, c] = x[tok_c, dc*128+dp]
    #   sc  : [nw, 128, 24] f32 scales:
    #         sc[j, dp, dc]    = sgu[e, dc*128+dp]   (dc in 0..7)
    #         sc[j, hl, 8+hc]  = swd[e, hc*128+hl]   (hc in 0..15)
    #   wgu8: [nw, 4, 128, 8192] i8  [fc, dp, dc*1024 + g*512 + jj]
    #         = q(Wgu)[e, g, fc*512+jj, dc*128+dp]
    #   wd8 : [nw, 2, 128, 8192] i8  [hh, hl, hcl*1024 + d]
    #         = q(Wd)[e, d, (hh*8+hcl)*128 + hl]
    #   out : [nw, 128, 1024] f16 (upcast on host)
    xt_d = nc.dram_tensor("xt", [nw, 128, 8, cap], f16, kind="ExternalInput").ap()
    sc_d = nc.dram_tensor("sc", [nw, 128, 24], f32, kind="ExternalInput").ap()
    wgu_d = nc.dram_tensor(
        "wgu8", [nw, 4, 128, 8192], i8, kind="ExternalInput"
    ).ap()
    wd_d = nc.dram_tensor("wd8", [nw, 2, 128, 8192], i8, kind="ExternalInput").ap()
    out_d = nc.dram_tensor("out", [nw, cap, 1024], f16, kind="ExternalOutput").ap()

    silu = mybir.ActivationFunctionType.Silu
    copy_f = mybir.ActivationFunctionType.Copy

    with tile.TileContext(nc) as tc:
        with (
            tc.tile_pool(name="xt_p", bufs=2) as xt_p,
            tc.tile_pool(name="sc_p", bufs=2) as sc_p,
            tc.tile_pool(name="wgu8_p", bufs=3) as wgu8_p,
            tc.tile_pool(name="wgu16_p", bufs=2) as wgu16_p,
            tc.tile_pool(name="wd8_p", bufs=3) as wd8_p,
            tc.tile_pool(name="wd16_p", bufs=2) as wd16_p,
            tc.tile_pool(name="sg_p", bufs=2) as sg_p,
            tc.tile_pool(name="intc_p", bufs=2) as intc_p,
            tc.tile_pool(name="intT_p", bufs=2) as intT_p,
            tc.tile_pool(name="o_p", bufs=2) as o_p,
            tc.tile_pool(name="id_p", bufs=1) as id_p,
            tc.tile_pool(name="ps_gu", bufs=2, space="PSUM") as ps_gu_p,
            tc.tile_pool(name="ps_tr", bufs=2, space="PSUM") as ps_tr_p,
            tc.tile_pool(name="ps_dn", bufs=1, space="PSUM") as ps_dn_p,
        ):
            ident = id_p.tile([128, 128], f16)
            make_identity(nc, ident[:])

            # dequant engine dispatch: index 0 = DVE, 1 = ACT, 2 = Pool
            def deq_op(eng_i, dst, src, scale):
                if eng_i == 1:
                    nc.scalar.activation(dst, src, copy_f, scale=scale)
                elif eng_i == 2:
                    nc.gpsimd.tensor_scalar_mul(dst, src, scale)
                else:
                    nc.vector.tensor_scalar_mul(dst, src, scale)

            def slice_engine(split, k):
                if k < split[0]:
                    return 0
                if k < split[0] + split[1]:
                    return 1
                return 2

            def emit_body():
                # Units per item: gu chunks fc=0..3, then down halves hh=0..1.
                # Software-pipelined emission: loads 2 units ahead, dequant
                # 1 unit ahead (before the current unit's epilogue so the
                # DVE/ACT/Pool queues don't head-of-line-block), PE transpose
                # of gu unit u-1 after unit u's matmuls.
                units = []
                for j in [jj for _ in range(rep) for jj in range(nw)]:
                    units += [("gu", j, fc) for fc in range(4)]
                    units += [("d", j, hh) for hh in range(2)]
                st: dict = {}   # per-unit tiles
                it: dict = {}   # per-item tiles (keyed by first unit index)

                def item_of(ui):
                    return it[ui - ui % 6]

                def load(ui):
                    kind, j, k = units[ui]
                    if kind == "gu" and k == 0:
                        xt_sb = xt_p.tile([128, 8, cap], f16)
                        nc.sync.dma_start(out=xt_sb[:], in_=xt_d[j])
                        sc_sb = sc_p.tile([128, 24], f32)
                        nc.scalar.dma_start(out=sc_sb[:], in_=sc_d[j])
                        it[ui] = {
                            "xt": xt_sb,
                            "sc": sc_sb,
                            "intT": intT_p.tile([128, 16, cap], f16, name="intT"),
                        }
                    if kind == "gu":
                        t8 = wgu8_p.tile([128, 8192], i8)
                        ring = nc.sync if k in (0, 2) else nc.scalar
                        ring.dma_start(out=t8[:], in_=wgu_d[j, k])
                    else:
                        t8 = wd8_p.tile([128, 8192], i8)
                        ring = nc.sync if k == 0 else nc.scalar
                        ring.dma_start(out=t8[:], in_=wd_d[j, k])
                    st[ui] = {"t8": t8}

                def dequant_thunks(ui):
                    kind, j, k = units[ui]
                    if probe == "dma":
                        return []
                    t8 = st[ui]["t8"]
                    sc_sb = item_of(ui)["sc"]
                    thunks = []
                    if kind == "gu":
                        t16 = wgu16_p.tile([128, 8, 2, 512], f16)
                        for dc in range(8):
                            dst = t16[:, dc]
                            src = t8[:, dc * 1024 : (dc + 1) * 1024]
                            if not do_dequant:
                                dst, src = t16[:, dc, :, ::32], src[:, ::32]
                            s1 = sc_sb[:, dc : dc + 1]
                            ei = slice_engine(gu_split, dc)
                            thunks.append(
                                lambda e=ei, d=dst, s=src, sc_=s1: deq_op(e, d, s, sc_)
                            )
                    else:
                        t16 = wd16_p.tile([128, 8, 1024], f16)
                        for hcl in range(8):
                            dst = t16[:, hcl]
                            src = t8[:, hcl * 1024 : (hcl + 1) * 1024]
                            if not do_dequant:
                                dst, src = dst[:, ::32], src[:, ::32]
                            s1 = sc_sb[:, 8 + k * 8 + hcl : 9 + k * 8 + hcl]
                            ei = slice_engine(wd_split, hcl)
                            thunks.append(
                                lambda e=ei, d=dst, s=src, sc_=s1: deq_op(e, d, s, sc_)
                            )
                    st[ui]["t16"] = t16
                    return thunks

                def compute(ui):
                    """Emit the unit's matmuls + epilogue. Returns a thunk
                    emitting the PE transposes (gu units), to be emitted
                    after the NEXT unit's matmuls."""
                    kind, j, k = units[ui]
                    im = item_of(ui)
                    if probe == "dma":
                        if kind == "d" and k == 1:
                            o_sb = o_p.tile([cap, 1024], f16)
                            nc.vector.tensor_copy(o_sb[:, :1], im["xt"][:, 0, :1])
                            nc.scalar.dma_start(out=out_d[j], in_=o_sb[:])
                        return None
                    if probe == "deqonly":
                        if kind == "d" and k == 1:
                            o_sb = o_p.tile([cap, 1024], f16)
                            nc.vector.tensor_copy(o_sb[:], st[ui]["t16"][:, 0, :])
                            nc.scalar.dma_start(out=out_d[j], in_=o_sb[:])
                        return None
                    xt_sb = im["xt"]
                    if kind == "gu":
                        t16 = st[ui]["t16"]
                        # two 512-wide matmuls per dc: gate and up halves of
                        # one 2-bank psum tile (s3d3 ISA caps one MM at 512)
                        ps_gu = ps_gu_p.tile([128, 2, 512], f32)
                        for dc in range(8):
                            for g in range(2):
                                nc.tensor.matmul(
                                    ps_gu[:, g], xt_sb[:, dc, :], t16[:, dc, g],
                                    start=(dc == 0), stop=(dc == 7),
                                )
                        sg = sg_p.tile([128, 512], f32)
                        nc.scalar.activation(sg[:], ps_gu[:, 0], silu)
                        intc = intc_p.tile([128, 512], f16)
                        nc.vector.tensor_mul(intc[:], sg[:], ps_gu[:, 1])
                        intT = im["intT"]
                        fc = k

                        def tthunk():
                            ps_t = ps_tr_p.tile([128, 4, cap], f16)
                            for kk in range(4):
                                nc.tensor.transpose(
                                    ps_t[:, kk, :],
                                    intc[:, kk * 128 : (kk + 1) * 128],
                                    ident[:],
                                )
                            nc.vector.tensor_copy(
                                intT[:, fc * 4 : (fc + 1) * 4, :], ps_t[:]
                            )

                        return tthunk
                    else:
                        hh = k
                        t16 = st[ui]["t16"]
                        if hh == 0:
                            im["ps_o"] = ps_dn_p.tile([cap, 1024], f32, name="ps_o")
                        ps_o = im["ps_o"]
                        intT = im["intT"]
                        for hcl in range(8):
                            hc = hh * 8 + hcl
                            for nt in range(2):
                                nc.tensor.matmul(
                                    ps_o[:, nt * 512 : (nt + 1) * 512],
                                    intT[:, hc, :],
                                    t16[:, hcl, nt * 512 : (nt + 1) * 512],
                                    start=(hc == 0), stop=(hc == 15),
                                )
                        if hh == 1:
                            o_sb = o_p.tile([cap, 1024], f16)
                            nc.scalar.activation(o_sb[:], ps_o[:], copy_f)
                            nc.scalar.dma_start(out=out_d[j], in_=o_sb[:])
                        return None

                U = len(units)
                load(0)
                if U > 1:
                    load(1)
                tq0 = dequant_thunks(0)
                for t in tq0:
                    t()
                pend_T = None
                for u in range(U):
                    if u + 2 < U:
                        load(u + 2)
                    tq = dequant_thunks(u + 1) if u + 1 < U else []
                    T_new = compute(u)
                    if pend_T is not None:
                        pend_T()
                    pend_T = T_new
                    for t in tq:
                        t()
                if pend_T is not None:
                    pend_T()

            if loop_n is None:
                emit_body()
            else:
                with tc.For_i(0, loop_n, 1):
                    emit_body()

    nc.compile()
    return nc


def _get_program(nw: int, cap: int):
    if (nw, cap) not in _CACHE:
        _CACHE[(nw, cap)] = _build(nw, cap)
    return _CACHE[(nw, cap)]


def _prepare(x, expert_indices, Wgu, Wd, cap_override=None):
    """Host dispatch + quantization + layout rearrangement."""
    x = np.ascontiguousarray(np.asarray(x), dtype=np.float32)
    ei = np.asarray(expert_indices).astype(np.int64)
    Wgu = np.ascontiguousarray(np.asarray(Wgu), dtype=np.float32)
    Wd = np.ascontiguousarray(np.asarray(Wd), dtype=np.float32)

    # ---- host dispatch: group (t, k) slots by expert ----
    flat = ei.ravel()  # slot s = t*K + k
    order = np.argsort(flat, kind="stable")
    counts = np.bincount(flat, minlength=E)
    offs = np.concatenate(([0], np.cumsum(counts)))
    slots_e = [order[offs[e] : offs[e + 1]] for e in range(E)]

    cap = C if cap_override is None else cap_override

    # work items per core: (expert, token slots) with <= cap tokens each
    items = [[] for _ in range(NCORES)]
    for e in range(E):
        c = e // EPC
        s = slots_e[e]
        for b in range(max(1, -(-len(s) // cap))):
            items[c].append((e, s[b * cap : (b + 1) * cap]))
    nw = max(len(it) for it in items)
    for c in range(NCORES):
        while len(items[c]) < nw:
            items[c].append((c * EPC, np.empty(0, np.int64)))

    # ---- host quantization (int8 symmetric, per-channel) ----
    # Wgu: scale per (e, d) over the (g, h) axes.
    sgu = np.abs(Wgu).max(axis=(1, 2)) / 127.0               # (E, D)
    qgu = np.clip(np.round(Wgu / sgu[:, None, None, :]), -127, 127).astype(
        np.int8
    )                                                         # (E, 2, H, D)
    # Wd: scale per (e, h) over d.
    swd = np.abs(Wd).max(axis=1) / 127.0                      # (E, H)
    qd = np.clip(np.round(Wd / swd[:, None, :]), -127, 127).astype(
        np.int8
    )                                                         # (E, D, H)

    # ---- layout rearrangement (partition-major) ----
    # wgu8_all[e, fc, dp, dc*1024 + g*512 + jj] = qgu[e, g, fc*512+jj, dc*128+dp]
    wgu8_all = (
        qgu.reshape(E, 2, 4, 512, 8, 128)   # e, g, fc, jj, dc, dp
        .transpose(0, 2, 5, 4, 1, 3)        # e, fc, dp, dc, g, jj
        .reshape(E, 4, 128, 8192)
    )
    # wd8_all[e, hh, hl, hcl*1024 + d] = qd[e, d, (hh*8+hcl)*128+hl]
    wd8_all = (
        qd.reshape(E, 1024, 2, 8, 128)      # e, d, hh, hcl, hl
        .transpose(0, 2, 4, 3, 1)           # e, hh, hl, hcl, d
        .reshape(E, 2, 128, 8192)
    )
    # sc_all[e, dp, 0:8] = sgu[e, dc*128+dp]; sc_all[e, hl, 8+hc] = swd[e, hc*128+hl]
    sc_all = np.zeros((E, 128, 24), np.float32)
    sc_all[:, :, :8] = sgu.reshape(E, 8, 128).transpose(0, 2, 1)
    sc_all[:, :, 8:24] = swd.reshape(E, 16, 128).transpose(0, 2, 1)

    xf = x.astype(np.float16)

    in_maps = []
    for c in range(NCORES):
        xt_h = np.zeros((nw, 128, 8, cap), np.float16)
        eids = np.array([e for e, _ in items[c]])
        for idx, (e, slots) in enumerate(items[c]):
            n = len(slots)
            if n:
                blk = np.zeros((cap, D), np.float16)
                blk[:n] = xf[slots // K]
                xt_h[idx] = blk.T.reshape(8, 128, cap).transpose(1, 0, 2)
        in_maps.append(
            {
                "xt": xt_h,
                "sc": np.ascontiguousarray(sc_all[eids]),
                "wgu8": np.ascontiguousarray(wgu8_all[eids]),
                "wd8": np.ascontiguousarray(wd8_all[eids]),
            }
        )
    return in_maps, items, nw, cap


def _combine(results, items):
    out = np.zeros((T * K, D), np.float32)
    for c in range(NCORES):
        o_core = results[c]["out"]  # (nw, cap, 1024) fp16
        for idx, (e, slots) in enumerate(items[c]):
            n = len(slots)
            if n:
                out[slots] = o_core[idx, :n].astype(np.float32)
    return out.reshape(T, K, D)


def kernel(x, expert_indices, Wgu, Wd):
    from concourse.bass_utils import run_bass_kernel_spmd

    in_maps, items, nw, cap = _prepare(x, expert_indices, Wgu, Wd)
    nc = _get_program(nw, cap)
    r = run_bass_kernel_spmd(nc, in_maps, list(range(NCORES)))
    kernel.last_results = r
    return _combine(r.results, items)
